# revision 14
# baseline (speedup 1.0000x reference)
"""CondConv (MoE routed conv) Trainium2 Bass kernel.

Strategy (8 NeuronCores, data-parallel over batch, 2 samples/core):
  - Routing on device: GAP reduce, linear on PE, sigmoid on ACT.
  - Per-sample combined weights cw[s] = sum_e r[s,e]*W[e]:
      * ot=0 half on the PE (diagonal trick, exact PSUM fp32 accumulation)
      * ot=1 half on DVE (s0) and GPSIMD (s1), hidden under the ot=0 conv
  - cw is cout-half-major; the weight stream is ordered so conv(s0, ot0)
    can start after only x(s0) + the ot0 slabs.
  - 3x3 conv = 18 accumulating f32r matmuls per [128 x 8 x 56] PSUM tile.
  - BN+SiLU fused in one ACT activation per tile; outputs DMA'd from ACT ring.
  - Junk fp32 matmuls at the start hold the PE HAM clock-gate at 2.4 GHz.
"""

import sys

sys.path.insert(0, "/opt/trn_rl_repo")

import numpy as np

import concourse.bass as bass  # noqa: F401
import concourse.mybir as mybir
import concourse.tile as tile
from concourse import bacc
from concourse.bass_utils import run_bass_kernel_spmd

F32 = mybir.dt.float32
F32R = mybir.dt.float32r
AF = mybir.ActivationFunctionType
ALU = mybir.AluOpType

B, CIN, H, W = 16, 256, 56, 56
E, COUT, KS = 8, 256, 3
NCORES = 8
SPC = B // NCORES
IT = CIN // 128
OT = COUT // 128
KHKW = KS * KS
HB = 8  # 7 h-blocks of 8 rows, N = 448
WP = W + 2
PIX = H * W
BN_EPS = 1e-5
SLAB = KHKW * 128  # 1152
CHUNK = 384
NCH = SLAB // CHUNK
NPA = 5  # phase-A open PSUM groups (= psc pool size)

_PROGRAM_CACHE = {}


def _build_program():
    nc = bacc.Bacc("TRN2", target_bir_lowering=False, debug=False)

    x_d = nc.dram_tensor("x", [SPC, IT, 128, H, WP], F32R, kind="ExternalInput")
    wt_d = nc.dram_tensor("wt", [E, OT, IT, 128, SLAB], F32R, kind="ExternalInput")
    rwt_d = nc.dram_tensor("rwt", [IT, 128, E], F32, kind="ExternalInput")
    rb_d = nc.dram_tensor("rb", [1, E], F32, kind="ExternalInput")
    ident_d = nc.dram_tensor("ident", [128, 128], F32, kind="ExternalInput")
    bns_d = nc.dram_tensor("bns", [OT, 128, 1], F32, kind="ExternalInput")
    bnb_d = nc.dram_tensor("bnb", [OT, 128, 1], F32, kind="ExternalInput")
    y_d = nc.dram_tensor("y", [SPC, OT, 128, H, W], F32, kind="ExternalOutput")

    with tile.TileContext(nc) as tc:
        with (
            tc.tile_pool(name="xp", bufs=1) as xp,
            tc.tile_pool(name="cwp", bufs=1) as cwp,
            tc.tile_pool(name="wtp", bufs=16) as wtp,
            tc.tile_pool(name="outp", bufs=4) as outp,
            tc.tile_pool(name="smal", bufs=1) as smal,
            tc.tile_pool(name="psc", bufs=NPA, space="PSUM") as psc,
            tc.tile_pool(name="psk", bufs=2, space="PSUM") as psk,
            tc.tile_pool(name="pss", bufs=1, space="PSUM") as pss,
        ):
            # ---- DMA order on sync ring: ident, x_s0, ot0 slabs, x_s1, ot1 ----
            ident_sb = smal.tile([128, 128], F32, tag="ident")
            nc.sync.dma_start(ident_sb[:], ident_d[:])

            x_sb = {}

            def load_x(s):
                for it in range(IT):
                    t = xp.tile(
                        [128, H, WP], F32R, tag=f"x_{s}_{it}", name=f"x_{s}_{it}"
                    )
                    nc.sync.dma_start(t[:], x_d[s, it])
                    x_sb[s, it] = t

            slab_tiles = {}

            def load_slabs(ot, its):
                for it in its:
                    for e in range(E):
                        wt_t = wtp.tile(
                            [128, SLAB], F32R, tag="wt", name=f"wt{ot}{it}{e}"
                        )
                        nc.sync.dma_start(wt_t[:], wt_d[e, ot, it])
                        slab_tiles[ot, it, e] = wt_t

            load_x(0)
            load_slabs(0, [0])
            load_x(1)
            load_slabs(0, [1])
            load_slabs(1, range(IT))

            # small loads on the SWDGE ring
            rwt_sb = []
            for it in range(IT):
                t = smal.tile([128, E], F32, tag=f"rwt{it}", name=f"rwt{it}")
                nc.gpsimd.dma_start(t[:], rwt_d[it])
                rwt_sb.append(t)
            rb_sb = smal.tile([1, E], F32, tag="rb")
            nc.gpsimd.dma_start(rb_sb[:], rb_d[:])
            bns_sb, bnb_sb = [], []
            for ot in range(OT):
                ts_ = smal.tile([128, 1], F32, tag=f"bns{ot}", name=f"bns{ot}")
                nc.gpsimd.dma_start(ts_[:], bns_d[ot])
                bns_sb.append(ts_)
                tb_ = smal.tile([128, 1], F32, tag=f"bnb{ot}", name=f"bnb{ot}")
                nc.gpsimd.dma_start(tb_[:], bnb_d[ot])
                bnb_sb.append(tb_)
            ones_sb = smal.tile([1, 128], F32, tag="ones")
            nc.vector.memset(ones_sb[:], 1.0)

            def warmup(n):
                # junk fp32 matmuls keep the PE HAM clock-gate at K=8/8
                for _ in range(n):
                    wps = psk.tile([128, CHUNK], F32, tag="kps", name="wps")
                    nc.tensor.matmul(
                        wps[:, 0:128], ident_sb[:], ident_sb[:], start=True, stop=True
                    )

            # ---- routing pieces ----
            pooled = {}
            rrow = {}
            r_bcast = {}
            diag = {}

            def routing_reduce_dve(s):
                for it in range(IT):
                    p = smal.tile(
                        [128, 1], F32, tag=f"pool{s}{it}", name=f"pool{s}{it}"
                    )
                    nc.vector.reduce_sum(
                        p[:],
                        x_sb[s, it][:].rearrange("p a b -> p (a b)"),
                        axis=mybir.AxisListType.X,
                    )
                    pooled[s, it] = p

            def routing_reduce_act(s, it):
                # in-place ACT copy with accum_out: frees the DVE, runs late
                p = smal.tile([128, 1], F32, tag=f"pool{s}{it}", name=f"pool{s}{it}")
                flat = x_sb[s, it][:].rearrange("p a b -> p (a b)")
                nc.scalar.activation(flat, flat, AF.Copy, accum_out=p[:])
                pooled[s, it] = p

            def routing_logits_pe(s):
                lg_ps = pss.tile([1, E], F32, tag="rps", name=f"lgps{s}")
                for it in range(IT):
                    nc.tensor.matmul(
                        lg_ps[:], pooled[s, it][:], rwt_sb[it][:],
                        start=(it == 0), stop=(it == IT - 1),
                    )
                return lg_ps

            def routing_z(s, lg_ps, eng):
                zr = smal.tile([1, E], F32, tag=f"z{s}", name=f"z{s}")
                eng.scalar_tensor_tensor(
                    zr[:], lg_ps[:], 1.0 / PIX, rb_sb[:], ALU.mult, ALU.add
                )
                rr = smal.tile([1, E], F32, tag=f"r{s}", name=f"r{s}")
                nc.scalar.activation(rr[:], zr[:], AF.Sigmoid)
                rrow[s] = rr

            def routing_bcast_pe(s, eng):
                rb_ps = pss.tile([128, E], F32, tag="rps", name=f"rbps{s}")
                nc.tensor.matmul(rb_ps[:], ones_sb[:], rrow[s][:], start=True, stop=True)
                rbc = smal.tile([128, E], F32, tag=f"rbc{s}", name=f"rbc{s}")
                eng.tensor_copy(rbc[:], rb_ps[:])
                r_bcast[s] = rbc

            def make_diag(s):
                for e in range(E):
                    dt_ = smal.tile(
                        [128, 128], F32R, tag=f"diag{s}{e}", name=f"diag{s}{e}"
                    )
                    nc.scalar.activation(
                        dt_[:], ident_sb[:], AF.Copy,
                        scale=r_bcast[s][:, e : e + 1],
                    )
                    diag[s, e] = dt_

            cw_r = {
                (s, it, ot): cwp.tile(
                    [128, SLAB], F32R,
                    tag=f"cwr_{s}_{it}_{ot}", name=f"cwr_{s}_{it}_{ot}",
                )
                for s in range(SPC)
                for it in range(IT)
                for ot in range(OT)
            }

            hblocks = [(h0, min(HB, H - h0)) for h0 in range(0, H, HB)]
            taps = [(0, 0)] + [
                (dh, dw) for dh in (-1, 0, 1) for dw in (-1, 0, 1) if (dh, dw) != (0, 0)
            ]

            def combine_pe_it(ot, it, s):
                # accumulate cw[s, it, ot] on the PE via the diagonal trick
                for c in range(NCH):
                    kps = psk.tile([128, CHUNK], F32, tag="kps", name="kps")
                    for e in range(E):
                        nc.tensor.matmul(
                            kps[:],
                            diag[s, e][:],
                            slab_tiles[ot, it, e][:, c * CHUNK : (c + 1) * CHUNK],
                            start=(e == 0),
                            stop=(e == E - 1),
                        )
                    nc.scalar.activation(
                        cw_r[s, it, ot][:, c * CHUNK : (c + 1) * CHUNK],
                        kps[:],
                        AF.Copy,
                    )

            def combine_vec(ot, eng):
                # f32r multiply-accumulate chains, slab-arrival-major so both
                # samples' reads of a slab happen back-to-back (frees the slot)
                for it in range(IT):
                    for e in range(E):
                        wt_t = slab_tiles[ot, it, e]
                        for s in range(SPC):
                            dst = cw_r[s, it, ot]
                            sc = r_bcast[s][:, e : e + 1]
                            if e == 0:
                                eng.tensor_scalar_mul(dst[:], wt_t[:], sc)
                            else:
                                eng.scalar_tensor_tensor(
                                    dst[:], wt_t[:], sc, dst[:], ALU.mult, ALU.add
                                )

            def block_total(h0, nh):
                return IT * sum(
                    1 for dh, dw in taps if min(h0 + nh, H - dh) > max(h0, -dh)
                )

            def conv_block_taps(ot, s, h0, nh, ps_t, its, n_mm, total):
                for it in its:
                    for dh, dw in taps:
                        khkw = (dh + 1) * 3 + (dw + 1)
                        ho_s = max(h0, -dh)
                        ho_e = min(h0 + nh, H - dh)
                        if ho_e <= ho_s:
                            continue
                        nhh = ho_e - ho_s
                        hi_s = ho_s + dh
                        off = khkw * 128
                        lhsT = cw_r[s, it, ot][:, off : off + 128]
                        rhs = x_sb[s, it][:, hi_s : hi_s + nhh, 1 + dw : 1 + dw + W]
                        out = ps_t[:, ho_s - h0 : ho_s - h0 + nhh, 0:W]
                        nc.tensor.matmul(
                            out, lhsT, rhs,
                            start=(n_mm == 0), stop=(n_mm == total - 1),
                        )
                        n_mm += 1
                return n_mm

            def conv_epilogue(ot, s, h0, nh, ps_t):
                o_t = outp.tile([128, HB, W], F32, tag="out", name="o_t")
                nc.scalar.activation(
                    o_t[:, :nh, :], ps_t[:, :nh, :], AF.Silu,
                    bias=bnb_sb[ot][:], scale=bns_sb[ot][:],
                )
                nc.scalar.dma_start(y_d[s, ot, :, h0 : h0 + nh, :], o_t[:, :nh, :])

            def conv_full_block(ot, s, h0, nh):
                ps_t = psc.tile([128, HB, W], F32, tag="ps", name="ps")
                total = block_total(h0, nh)
                n_mm = conv_block_taps(ot, s, h0, nh, ps_t, range(IT), 0, total)
                assert n_mm == total
                conv_epilogue(ot, s, h0, nh, ps_t)

            # ================= emission schedule =================
            warmup(50)
            routing_reduce_dve(0)
            lg0 = routing_logits_pe(0)
            routing_z(0, lg0, nc.vector)
            routing_bcast_pe(0, nc.vector)
            make_diag(0)
            warmup(10)

            # combine ot0/s0 on PE, phase-A conv (it0 taps) between the halves
            pa = []
            for it in range(IT):
                combine_pe_it(0, it, 0)
                if it == 0:
                    routing_reduce_act(1, 0)
                    routing_reduce_act(1, 1)
                    for h0, nh in hblocks[:NPA]:
                        ps_t = psc.tile([128, HB, W], F32, tag="ps", name="ps")
                        total = block_total(h0, nh)
                        n_mm = conv_block_taps(0, 0, h0, nh, ps_t, [0], 0, total)
                        pa.append((h0, nh, ps_t, n_mm, total))

            # s1 routing tail + combine ot0/s1 on PE as early as possible so
            # the ot0 slab pool slots free up for the ot1 stream
            lg1 = routing_logits_pe(1)
            routing_z(1, lg1, nc.vector)
            routing_bcast_pe(1, nc.vector)
            make_diag(1)
            for it in range(IT):
                combine_pe_it(0, it, 1)

            # combine ot1 on the DVE (slab-paced, hidden under conv ot0)
            combine_vec(1, nc.vector)

            # phase-B: close phase-A blocks
            for h0, nh, ps_t, n_mm, total in pa:
                n_mm = conv_block_taps(0, 0, h0, nh, ps_t, [1], n_mm, total)
                assert n_mm == total
                conv_epilogue(0, 0, h0, nh, ps_t)

            # remaining s0/ot0 blocks, conv s1/ot0, conv ot1
            for h0, nh in hblocks[NPA:]:
                conv_full_block(0, 0, h0, nh)
            for h0, nh in hblocks:
                conv_full_block(0, 1, h0, nh)
            for s in range(SPC):
                for h0, nh in hblocks:
                    conv_full_block(1, s, h0, nh)

    nc.compile()
    return nc


def _get_program():
    if "nc" not in _PROGRAM_CACHE:
        _PROGRAM_CACHE["nc"] = _build_program()
    return _PROGRAM_CACHE["nc"]


def kernel(x, routing_w, routing_b, kernel_weights, bn_gamma, bn_beta, bn_mean, bn_var,
           _trace=False, _trace_kwargs=None):
    x = np.asarray(x, dtype=np.float32)
    routing_w = np.asarray(routing_w, dtype=np.float32)
    routing_b = np.asarray(routing_b, dtype=np.float32)
    kernel_weights = np.asarray(kernel_weights, dtype=np.float32)
    bn_gamma = np.asarray(bn_gamma, dtype=np.float32)
    bn_beta = np.asarray(bn_beta, dtype=np.float32)
    bn_mean = np.asarray(bn_mean, dtype=np.float32)
    bn_var = np.asarray(bn_var, dtype=np.float32)

    # wt[e, ot, it, i, khkw*128 + o_in] from kernel_weights[e, o, i, kh, kw]
    kw7 = kernel_weights.reshape(E, OT, 128, IT, 128, KS, KS)
    wt_host = np.ascontiguousarray(kw7.transpose(0, 1, 3, 4, 5, 6, 2)).reshape(
        E, OT, IT, 128, SLAB
    )
    rwt_host = np.ascontiguousarray(routing_w.T).reshape(IT, 128, E)
    rb_host = np.ascontiguousarray(routing_b).reshape(1, E)
    ident_host = np.eye(128, dtype=np.float32)
    inv = bn_gamma / np.sqrt(bn_var + BN_EPS)
    bns_host = np.ascontiguousarray(inv).reshape(OT, 128, 1)
    bnb_host = np.ascontiguousarray(bn_beta - bn_mean * inv).reshape(OT, 128, 1)

    x_pad = np.zeros((B, CIN, H, WP), dtype=np.float32)
    x_pad[:, :, :, 1 : 1 + W] = x
    in_maps = []
    for g in range(NCORES):
        xg = np.ascontiguousarray(
            x_pad[g * SPC : (g + 1) * SPC].reshape(SPC, IT, 128, H, WP)
        )
        in_maps.append(
            {
                "x": xg,
                "wt": wt_host,
                "rwt": rwt_host,
                "rb": rb_host,
                "ident": ident_host,
                "bns": bns_host,
                "bnb": bnb_host,
            }
        )

    nc = _get_program()
    res = run_bass_kernel_spmd(
        nc, in_maps, core_ids=list(range(NCORES)),
        trace=_trace, **(_trace_kwargs or {}),
    )
    _PROGRAM_CACHE["last_result"] = res

    out = np.empty((B, COUT, H, W), dtype=np.float32)
    for g in range(NCORES):
        yg = res.results[g]["y"]
        out[g * SPC : (g + 1) * SPC] = yg.reshape(SPC, COUT, H, W)
    return out


# revision 15
# speedup vs baseline: 1.1414x; 1.1414x over previous
"""CondConv (MoE routed conv) Trainium2 Bass kernel.

Strategy (8 NeuronCores, data-parallel over batch, 2 samples/core):
  - Routing on device: GAP reduce, linear on PE, sigmoid on ACT.
  - Per-sample combined weights cw[s] = sum_e r[s,e]*W[e]:
      * ot=0 half on the PE (diagonal trick, exact PSUM fp32 accumulation)
      * ot=1 half on DVE (s0) and GPSIMD (s1), hidden under the ot=0 conv
  - cw is cout-half-major; the weight stream is ordered so conv(s0, ot0)
    can start after only x(s0) + the ot0 slabs.
  - 3x3 conv = 18 accumulating f32r matmuls per [128 x 8 x 56] PSUM tile.
  - BN+SiLU fused in one ACT activation per tile; outputs DMA'd from ACT ring.
  - Junk fp32 matmuls at the start hold the PE HAM clock-gate at 2.4 GHz.
"""

import sys

sys.path.insert(0, "/opt/trn_rl_repo")

import numpy as np

import concourse.bass as bass  # noqa: F401
import concourse.mybir as mybir
import concourse.tile as tile
from concourse import bacc
from concourse.bass_utils import run_bass_kernel_spmd

F32 = mybir.dt.float32
F32R = mybir.dt.float32r
AF = mybir.ActivationFunctionType
ALU = mybir.AluOpType

B, CIN, H, W = 16, 256, 56, 56
E, COUT, KS = 8, 256, 3
NCORES = 8
SPC = B // NCORES
IT = CIN // 128
OT = COUT // 128
KHKW = KS * KS
HB = 8  # 7 h-blocks of 8 rows, N = 448
WP = W + 2
PIX = H * W
BN_EPS = 1e-5
SLAB = KHKW * 128  # 1152
CHUNK = 384
NCH = SLAB // CHUNK
NPA = 5  # phase-A open PSUM groups (= psc pool size)

_PROGRAM_CACHE = {}


def _build_program():
    nc = bacc.Bacc("TRN2", target_bir_lowering=False, debug=False)

    x_d = nc.dram_tensor("x", [SPC, IT, 128, H, WP], F32R, kind="ExternalInput")
    wt_d = nc.dram_tensor("wt", [E, OT, IT, 128, SLAB], F32R, kind="ExternalInput")
    rwt_d = nc.dram_tensor("rwt", [IT, 128, E], F32, kind="ExternalInput")
    rb_d = nc.dram_tensor("rb", [1, E], F32, kind="ExternalInput")
    ident_d = nc.dram_tensor("ident", [128, 128], F32, kind="ExternalInput")
    bns_d = nc.dram_tensor("bns", [OT, 128, 1], F32, kind="ExternalInput")
    bnb_d = nc.dram_tensor("bnb", [OT, 128, 1], F32, kind="ExternalInput")
    y_d = nc.dram_tensor("y", [SPC, OT, 128, H, W], F32, kind="ExternalOutput")

    with tile.TileContext(nc) as tc:
        with (
            tc.tile_pool(name="xp", bufs=1) as xp,
            tc.tile_pool(name="cwp", bufs=1) as cwp,
            tc.tile_pool(name="wtp", bufs=16) as wtp,
            tc.tile_pool(name="outp", bufs=4) as outp,
            tc.tile_pool(name="smal", bufs=1) as smal,
            tc.tile_pool(name="psc", bufs=NPA, space="PSUM") as psc,
            tc.tile_pool(name="psk", bufs=2, space="PSUM") as psk,
            tc.tile_pool(name="pss", bufs=1, space="PSUM") as pss,
        ):
            # ---- DMA order on sync ring: ident, x_s0, ot0 slabs, x_s1, ot1 ----
            ident_sb = smal.tile([128, 128], F32, tag="ident")
            nc.sync.dma_start(ident_sb[:], ident_d[:])

            x_sb = {}

            def load_x(s):
                for it in range(IT):
                    t = xp.tile(
                        [128, H, WP], F32R, tag=f"x_{s}_{it}", name=f"x_{s}_{it}"
                    )
                    nc.sync.dma_start(t[:], x_d[s, it])
                    x_sb[s, it] = t

            slab_tiles = {}

            def load_slabs(ot, its):
                for it in its:
                    for e in range(E):
                        wt_t = wtp.tile(
                            [128, SLAB], F32R, tag="wt", name=f"wt{ot}{it}{e}"
                        )
                        nc.sync.dma_start(wt_t[:], wt_d[e, ot, it])
                        slab_tiles[ot, it, e] = wt_t

            load_x(0)
            load_slabs(0, [0])
            load_x(1)
            load_slabs(0, [1])
            load_slabs(1, range(IT))

            # small loads on the SWDGE ring
            rwt_sb = []
            for it in range(IT):
                t = smal.tile([128, E], F32, tag=f"rwt{it}", name=f"rwt{it}")
                nc.gpsimd.dma_start(t[:], rwt_d[it])
                rwt_sb.append(t)
            rb_sb = smal.tile([1, E], F32, tag="rb")
            nc.gpsimd.dma_start(rb_sb[:], rb_d[:])
            bns_sb, bnb_sb = [], []
            for ot in range(OT):
                ts_ = smal.tile([128, 1], F32, tag=f"bns{ot}", name=f"bns{ot}")
                nc.gpsimd.dma_start(ts_[:], bns_d[ot])
                bns_sb.append(ts_)
                tb_ = smal.tile([128, 1], F32, tag=f"bnb{ot}", name=f"bnb{ot}")
                nc.gpsimd.dma_start(tb_[:], bnb_d[ot])
                bnb_sb.append(tb_)
            ones_sb = smal.tile([1, 128], F32, tag="ones")
            nc.vector.memset(ones_sb[:], 1.0)

            def warmup(n):
                # junk fp32 matmuls keep the PE HAM clock-gate at K=8/8
                for _ in range(n):
                    wps = psk.tile([128, CHUNK], F32, tag="kps", name="wps")
                    nc.tensor.matmul(
                        wps[:, 0:128], ident_sb[:], ident_sb[:], start=True, stop=True
                    )

            # ---- routing pieces ----
            pooled = {}
            rrow = {}
            r_bcast = {}
            diag = {}

            def routing_reduce_dve(s):
                for it in range(IT):
                    p = smal.tile(
                        [128, 1], F32, tag=f"pool{s}{it}", name=f"pool{s}{it}"
                    )
                    nc.vector.reduce_sum(
                        p[:],
                        x_sb[s, it][:].rearrange("p a b -> p (a b)"),
                        axis=mybir.AxisListType.X,
                    )
                    pooled[s, it] = p

            def routing_reduce_act(s, it):
                # in-place ACT copy with accum_out: frees the DVE, runs late
                p = smal.tile([128, 1], F32, tag=f"pool{s}{it}", name=f"pool{s}{it}")
                flat = x_sb[s, it][:].rearrange("p a b -> p (a b)")
                nc.scalar.activation(flat, flat, AF.Copy, accum_out=p[:])
                pooled[s, it] = p

            def routing_logits_pe(s):
                lg_ps = pss.tile([1, E], F32, tag="rps", name=f"lgps{s}")
                for it in range(IT):
                    nc.tensor.matmul(
                        lg_ps[:], pooled[s, it][:], rwt_sb[it][:],
                        start=(it == 0), stop=(it == IT - 1),
                    )
                return lg_ps

            def routing_z(s, lg_ps, eng):
                zr = smal.tile([1, E], F32, tag=f"z{s}", name=f"z{s}")
                eng.scalar_tensor_tensor(
                    zr[:], lg_ps[:], 1.0 / PIX, rb_sb[:], ALU.mult, ALU.add
                )
                rr = smal.tile([1, E], F32, tag=f"r{s}", name=f"r{s}")
                nc.scalar.activation(rr[:], zr[:], AF.Sigmoid)
                rrow[s] = rr

            def routing_bcast_pe(s, eng):
                rb_ps = pss.tile([128, E], F32, tag="rps", name=f"rbps{s}")
                nc.tensor.matmul(rb_ps[:], ones_sb[:], rrow[s][:], start=True, stop=True)
                rbc = smal.tile([128, E], F32, tag=f"rbc{s}", name=f"rbc{s}")
                eng.tensor_copy(rbc[:], rb_ps[:])
                r_bcast[s] = rbc

            def make_diag(s):
                for e in range(E):
                    dt_ = smal.tile(
                        [128, 128], F32R, tag=f"diag{s}{e}", name=f"diag{s}{e}"
                    )
                    nc.scalar.activation(
                        dt_[:], ident_sb[:], AF.Copy,
                        scale=r_bcast[s][:, e : e + 1],
                    )
                    diag[s, e] = dt_

            cw_r = {
                (s, it, ot): cwp.tile(
                    [128, SLAB], F32R,
                    tag=f"cwr_{s}_{it}_{ot}", name=f"cwr_{s}_{it}_{ot}",
                )
                for s in range(SPC)
                for it in range(IT)
                for ot in range(OT)
            }

            hblocks = [(h0, min(HB, H - h0)) for h0 in range(0, H, HB)]
            taps = [(0, 0)] + [
                (dh, dw) for dh in (-1, 0, 1) for dw in (-1, 0, 1) if (dh, dw) != (0, 0)
            ]

            def combine_pe_it(ot, it, s):
                # accumulate cw[s, it, ot] on the PE via the diagonal trick
                for c in range(NCH):
                    kps = psk.tile([128, CHUNK], F32, tag="kps", name="kps")
                    for e in range(E):
                        nc.tensor.matmul(
                            kps[:],
                            diag[s, e][:],
                            slab_tiles[ot, it, e][:, c * CHUNK : (c + 1) * CHUNK],
                            start=(e == 0),
                            stop=(e == E - 1),
                        )
                    nc.scalar.activation(
                        cw_r[s, it, ot][:, c * CHUNK : (c + 1) * CHUNK],
                        kps[:],
                        AF.Copy,
                    )

            def combine_vec(ot, eng):
                # f32r multiply-accumulate chains, slab-arrival-major so both
                # samples' reads of a slab happen back-to-back (frees the slot)
                for it in range(IT):
                    for e in range(E):
                        wt_t = slab_tiles[ot, it, e]
                        for s in range(SPC):
                            dst = cw_r[s, it, ot]
                            sc = r_bcast[s][:, e : e + 1]
                            if e == 0:
                                eng.tensor_scalar_mul(dst[:], wt_t[:], sc)
                            else:
                                eng.scalar_tensor_tensor(
                                    dst[:], wt_t[:], sc, dst[:], ALU.mult, ALU.add
                                )

            def block_total(h0, nh):
                return IT * sum(
                    1 for dh, dw in taps if min(h0 + nh, H - dh) > max(h0, -dh)
                )

            def conv_block_taps(ot, s, h0, nh, ps_t, its, n_mm, total):
                for it in its:
                    for dh, dw in taps:
                        khkw = (dh + 1) * 3 + (dw + 1)
                        ho_s = max(h0, -dh)
                        ho_e = min(h0 + nh, H - dh)
                        if ho_e <= ho_s:
                            continue
                        nhh = ho_e - ho_s
                        hi_s = ho_s + dh
                        off = khkw * 128
                        lhsT = cw_r[s, it, ot][:, off : off + 128]
                        rhs = x_sb[s, it][:, hi_s : hi_s + nhh, 1 + dw : 1 + dw + W]
                        out = ps_t[:, ho_s - h0 : ho_s - h0 + nhh, 0:W]
                        nc.tensor.matmul(
                            out, lhsT, rhs,
                            start=(n_mm == 0), stop=(n_mm == total - 1),
                        )
                        n_mm += 1
                return n_mm

            def conv_epilogue(ot, s, h0, nh, ps_t):
                o_t = outp.tile([128, HB, W], F32, tag="out", name="o_t")
                nc.scalar.activation(
                    o_t[:, :nh, :], ps_t[:, :nh, :], AF.Silu,
                    bias=bnb_sb[ot][:], scale=bns_sb[ot][:],
                )
                nc.gpsimd.dma_start(y_d[s, ot, :, h0 : h0 + nh, :], o_t[:, :nh, :])

            def conv_full_block(ot, s, h0, nh):
                ps_t = psc.tile([128, HB, W], F32, tag="ps", name="ps")
                total = block_total(h0, nh)
                n_mm = conv_block_taps(ot, s, h0, nh, ps_t, range(IT), 0, total)
                assert n_mm == total
                conv_epilogue(ot, s, h0, nh, ps_t)

            # ================= emission schedule =================
            warmup(50)
            routing_reduce_dve(0)
            lg0 = routing_logits_pe(0)
            routing_z(0, lg0, nc.vector)
            routing_bcast_pe(0, nc.vector)
            make_diag(0)
            warmup(10)

            # combine ot0/s0 on PE, phase-A conv (it0 taps) between the halves
            pa = []
            for it in range(IT):
                combine_pe_it(0, it, 0)
                if it == 0:
                    routing_reduce_act(1, 0)
                    routing_reduce_act(1, 1)
                    for h0, nh in hblocks[:NPA]:
                        ps_t = psc.tile([128, HB, W], F32, tag="ps", name="ps")
                        total = block_total(h0, nh)
                        n_mm = conv_block_taps(0, 0, h0, nh, ps_t, [0], 0, total)
                        pa.append((h0, nh, ps_t, n_mm, total))

            # s1 routing tail + combine ot0/s1 on PE as early as possible so
            # the ot0 slab pool slots free up for the ot1 stream
            lg1 = routing_logits_pe(1)
            routing_z(1, lg1, nc.vector)
            routing_bcast_pe(1, nc.vector)
            make_diag(1)
            for it in range(IT):
                combine_pe_it(0, it, 1)

            # combine ot1 on the DVE (slab-paced, hidden under conv ot0)
            combine_vec(1, nc.vector)

            # phase-B: close phase-A blocks
            for h0, nh, ps_t, n_mm, total in pa:
                n_mm = conv_block_taps(0, 0, h0, nh, ps_t, [1], n_mm, total)
                assert n_mm == total
                conv_epilogue(0, 0, h0, nh, ps_t)

            # remaining s0/ot0 blocks, conv s1/ot0, conv ot1
            for h0, nh in hblocks[NPA:]:
                conv_full_block(0, 0, h0, nh)
            for h0, nh in hblocks:
                conv_full_block(0, 1, h0, nh)
            for s in range(SPC):
                for h0, nh in hblocks:
                    conv_full_block(1, s, h0, nh)

    nc.compile()
    return nc


def _get_program():
    if "nc" not in _PROGRAM_CACHE:
        _PROGRAM_CACHE["nc"] = _build_program()
    return _PROGRAM_CACHE["nc"]


def kernel(x, routing_w, routing_b, kernel_weights, bn_gamma, bn_beta, bn_mean, bn_var,
           _trace=False, _trace_kwargs=None):
    x = np.asarray(x, dtype=np.float32)
    routing_w = np.asarray(routing_w, dtype=np.float32)
    routing_b = np.asarray(routing_b, dtype=np.float32)
    kernel_weights = np.asarray(kernel_weights, dtype=np.float32)
    bn_gamma = np.asarray(bn_gamma, dtype=np.float32)
    bn_beta = np.asarray(bn_beta, dtype=np.float32)
    bn_mean = np.asarray(bn_mean, dtype=np.float32)
    bn_var = np.asarray(bn_var, dtype=np.float32)

    # wt[e, ot, it, i, khkw*128 + o_in] from kernel_weights[e, o, i, kh, kw]
    kw7 = kernel_weights.reshape(E, OT, 128, IT, 128, KS, KS)
    wt_host = np.ascontiguousarray(kw7.transpose(0, 1, 3, 4, 5, 6, 2)).reshape(
        E, OT, IT, 128, SLAB
    )
    rwt_host = np.ascontiguousarray(routing_w.T).reshape(IT, 128, E)
    rb_host = np.ascontiguousarray(routing_b).reshape(1, E)
    ident_host = np.eye(128, dtype=np.float32)
    inv = bn_gamma / np.sqrt(bn_var + BN_EPS)
    bns_host = np.ascontiguousarray(inv).reshape(OT, 128, 1)
    bnb_host = np.ascontiguousarray(bn_beta - bn_mean * inv).reshape(OT, 128, 1)

    x_pad = np.zeros((B, CIN, H, WP), dtype=np.float32)
    x_pad[:, :, :, 1 : 1 + W] = x
    in_maps = []
    for g in range(NCORES):
        xg = np.ascontiguousarray(
            x_pad[g * SPC : (g + 1) * SPC].reshape(SPC, IT, 128, H, WP)
        )
        in_maps.append(
            {
                "x": xg,
                "wt": wt_host,
                "rwt": rwt_host,
                "rb": rb_host,
                "ident": ident_host,
                "bns": bns_host,
                "bnb": bnb_host,
            }
        )

    nc = _get_program()
    res = run_bass_kernel_spmd(
        nc, in_maps, core_ids=list(range(NCORES)),
        trace=_trace, **(_trace_kwargs or {}),
    )
    _PROGRAM_CACHE["last_result"] = res

    out = np.empty((B, COUT, H, W), dtype=np.float32)
    for g in range(NCORES):
        yg = res.results[g]["y"]
        out[g * SPC : (g + 1) * SPC] = yg.reshape(SPC, COUT, H, W)
    return out
